# revision 9
# baseline (speedup 1.0000x reference)
"""CrossLocalAttentionLayer (GNN message passing) as a Bass/Tile kernel on 8 trn2 cores.

Strategy (data parallel over graphs + node-split within a graph):
  - 8 cores, B=4 graphs: core c handles graph c//2, node half c%2 (8192 nodes).
  - Edges are assigned to the core owning their src node, bucketed into 64
    balanced node-tiles of 128 nodes, padded to a uniform 9 sub-blocks of 128
    edges per node-tile (capacity 73728 edge slots per core).
  - On device, per core:
      phase 1: project K,V for the whole graph into a DRAM table [N,256].
      phase 2: per node-tile: project Q; per 1024-edge super-block, gather
        K/V rows by tgt with a hardware dma_gather; per 128-edge sub-block,
        project E, build the src one-hot selection matrix with is_equal
        against an iota matrix, gather Q rows with a PE matmul, compute
        attention logits/exp/messages with DVE+ACT, and accumulate
        numerator/denominator into PSUM with PE matmuls (segmented
        scatter-add as dense matmul against the selection matrix).
      finalize per node-tile: normalize, fc matmul, residual add, layernorm.
  - Host: bucketing/padding of indices, edge-feature redistribution in device
    order, inverse permutations on outputs.
"""

import os
import numpy as np
from contextlib import ExitStack
from dataclasses import dataclass

import concourse.bass as bass
import concourse.bacc as bacc
import concourse.mybir as mybir
import concourse.tile as tile
from concourse import bass_utils
from concourse.masks import make_identity

P = 128
F32 = mybir.dt.float32
I16 = mybir.dt.int16
AF = mybir.ActivationFunctionType
ALU = mybir.AluOpType

H, DK, DV, DM = 4, 32, 32, 128


@dataclass(frozen=True)
class Cfg:
    n_full: int = 16384   # nodes per graph
    nh: int = 8192        # nodes per core
    kb: int = 9           # 128-edge sub-blocks per node-tile
    ss: int = 8           # sub-blocks per gather super-block

    @property
    def nt(self):
        return self.nh // P

    @property
    def ecap(self):
        return self.nt * self.kb * P

    @property
    def se(self):
        return self.ss * P  # edges per super-block

    @property
    def nsup(self):
        return self.nt * self.kb // self.ss


FULL = Cfg()
N_CORES = 8
B_FULL, EG_FULL = 4, 131072


def build_program(cfg: Cfg, repeat: int = 1):
    """Build the single SPMD Bass program run on every core.

    repeat>1 duplicates the whole compute body (timing harness use only);
    outputs are simply overwritten on later repeats.
    """
    nc = bacc.Bacc("TRN2", target_bir_lowering=False, debug=False,
                   enable_asserts=False, num_devices=1)
    f32 = F32
    NF, NH, NT, KB, SS = cfg.n_full, cfg.nh, cfg.nt, cfg.kb, cfg.ss
    SE, NSUP = cfg.se, cfg.nsup

    din = {}
    def inp(name, shape, dt=f32):
        din[name] = nc.dram_tensor(name, shape, dt, kind="ExternalInput").ap()
        return din[name]

    xqT = inp("xqT", [P, NH])
    xq = inp("xq", [NH, P])
    xkT = inp("xkT", [P, NF])
    xvT = inp("xvT", [P, NF])
    ef = inp("ef", [NSUP, P, SS * P])
    tgt16 = inp("tgt16", [NSUP, P, SE // 16], I16)
    srcilv = inp("srcilv", [NT, P, KB])
    wq = inp("wq", [P, P])
    wk = inp("wk", [P, P])
    wv = inp("wv", [P, P])
    we = inp("we", [P, P])
    wfc = inp("wfc", [P, P])
    e4t = inp("e4t", [H, P])
    iota = inp("iota", [P])
    gamma = inp("gamma", [P])
    beta = inp("beta", [P])

    outp = nc.dram_tensor("outp", [NH, P], f32, kind="ExternalOutput").ap()
    attn_o = nc.dram_tensor("attn_o", [NT, P, KB * H], f32, kind="ExternalOutput").ap()
    kvtab = nc.dram_tensor("kvtab", [NF, 2 * P], f32, kind="Internal").ap()

    with tile.TileContext(nc) as tc, ExitStack() as ctx:
        consts = ctx.enter_context(tc.tile_pool(name="consts", bufs=1))
        p1 = ctx.enter_context(tc.tile_pool(name="p1", bufs=3))
        kvp = ctx.enter_context(tc.tile_pool(name="kvp", bufs=3))
        efp = ctx.enter_context(tc.tile_pool(name="efp", bufs=3))
        idxp = ctx.enter_context(tc.tile_pool(name="idxp", bufs=3))
        ntp = ctx.enter_context(tc.tile_pool(name="ntp", bufs=2))
        wkp = ctx.enter_context(tc.tile_pool(name="wkp", bufs=3))
        smallp = ctx.enter_context(tc.tile_pool(name="smallp", bufs=4))
        # PSUM pools: 8 banks total
        ps_acc = ctx.enter_context(tc.tile_pool(name="ps_acc", bufs=2, space="PSUM"))
        ps_den = ctx.enter_context(tc.tile_pool(name="ps_den", bufs=1, space="PSUM"))
        ps_ee = ctx.enter_context(tc.tile_pool(name="ps_ee", bufs=1, space="PSUM"))
        ps_selt = ctx.enter_context(tc.tile_pool(name="ps_selt", bufs=1, space="PSUM"))
        ps_qe = ctx.enter_context(tc.tile_pool(name="ps_qe", bufs=1, space="PSUM"))
        ps_nt = ctx.enter_context(tc.tile_pool(name="ps_nt", bufs=2, space="PSUM"))

        # ---- constants ----
        ident = consts.tile([P, P], f32)
        make_identity(nc, ident[:])
        iota_mat = consts.tile([P, P], f32)
        nc.sync.dma_start(iota_mat[:], iota[None, :].to_broadcast([P, P]))
        gamma_mat = consts.tile([P, P], f32)
        nc.sync.dma_start(gamma_mat[:], gamma[None, :].to_broadcast([P, P]))
        beta_mat = consts.tile([P, P], f32)
        nc.sync.dma_start(beta_mat[:], beta[None, :].to_broadcast([P, P]))
        w_sb = {}
        for name, d in (("wq", wq), ("wk", wk), ("wv", wv), ("we", we), ("wfc", wfc)):
            t = consts.tile([P, P], f32, tag=name)
            nc.sync.dma_start(t[:], d[:])
            w_sb[name] = t
        e4t_sb = consts.tile([H, P], f32)
        nc.sync.dma_start(e4t_sb[:], e4t[:])

        # ---- phase 1: K/V table ----
        def emit_phase1():
          for i in range(NF // P):
            xk_t = p1.tile([P, P], f32, tag="xk")
            nc.sync.dma_start(xk_t[:], xkT[:, i * P:(i + 1) * P])
            xv_t = p1.tile([P, P], f32, tag="xv")
            nc.sync.dma_start(xv_t[:], xvT[:, i * P:(i + 1) * P])
            kv_ps = ps_nt.tile([P, 2 * P], f32, tag="ntwork")
            nc.tensor.matmul(kv_ps[:, 0:P], lhsT=xk_t[:], rhs=w_sb["wk"][:],
                             start=True, stop=True)
            nc.tensor.matmul(kv_ps[:, P:2 * P], lhsT=xv_t[:], rhs=w_sb["wv"][:],
                             start=True, stop=True)
            kv_sb = p1.tile([P, 2 * P], f32, tag="kvsb")
            nc.vector.tensor_copy(kv_sb[:], kv_ps[:])
            nc.sync.dma_start(kvtab[i * P:(i + 1) * P, :], kv_sb[:])

        # ---- phase 2 ----
        super_tiles = {}

        def ensure_super(s):
            if s in super_tiles or s >= NSUP:
                return
            idx_t = idxp.tile([P, SE // 16], I16, tag="idx")
            nc.sync.dma_start(idx_t[:], tgt16[s])
            kv_t = kvp.tile([P, SS, 2 * P], f32, tag="kvg")
            nc.gpsimd.dma_gather(
                out_ap=kv_t[:], in_ap=kvtab[:], idxs_ap=idx_t[:],
                num_idxs=SE, num_idxs_reg=SE, elem_size=2 * P)
            ef_t = efp.tile([P, SS * P], f32, tag="ef")
            nc.sync.dma_start(ef_t[:], ef[s])
            super_tiles[s] = (kv_t, ef_t)

        def emit_phase2():
          for nt_i in range(NT):
            srcilv_t = ntp.tile([P, KB], f32, tag="srcilv")
            nc.sync.dma_start(srcilv_t[:], srcilv[nt_i])
            xqT_t = ntp.tile([P, P], f32, tag="xqT")
            nc.sync.dma_start(xqT_t[:], xqT[:, nt_i * P:(nt_i + 1) * P])
            q_ps = ps_nt.tile([P, P], f32, tag="ntwork")
            nc.tensor.matmul(q_ps[:], lhsT=xqT_t[:], rhs=w_sb["wq"][:],
                             start=True, stop=True)
            qtile = ntp.tile([P, P], f32, tag="qtile")
            nc.scalar.activation(qtile[:], q_ps[:], AF.Copy)
            xq_t = ntp.tile([P, P], f32, tag="xqres")
            nc.sync.dma_start(xq_t[:], xq[nt_i * P:(nt_i + 1) * P, :])
            attn_sb = ntp.tile([P, KB * H], f32, tag="attnsb")
            numT = ps_acc.tile([P, P], f32, tag="numT")
            denT = ps_den.tile([H, P], f32, tag="denT")

            for j in range(KB):
                sbi = nt_i * KB + j
                s, jj = divmod(sbi, SS)
                ensure_super(s)
                if jj == 0:
                    ensure_super(s + 1)
                kv_t, ef_t = super_tiles[s]
                ke = kv_t[:, jj, 0:P]
                ve = kv_t[:, jj, P:2 * P]

                e_ps = ps_ee.tile([P, P], f32, tag="ee")
                nc.tensor.matmul(e_ps[:], lhsT=ef_t[:, jj * P:(jj + 1) * P],
                                 rhs=w_sb["we"][:], start=True, stop=True)

                sel = wkp.tile([P, P], f32, tag="sel")
                nc.vector.tensor_tensor(
                    out=sel[:], in0=srcilv_t[:, j:j + 1].to_broadcast([P, P]),
                    in1=iota_mat[:], op=ALU.is_equal)
                selT_ps = ps_selt.tile([P, P], f32, tag="selt")
                nc.tensor.transpose(selT_ps[:], sel[:], ident[:])
                selT = wkp.tile([P, P], f32, tag="selT")
                nc.vector.tensor_copy(selT[:], selT_ps[:])

                qe_ps = ps_qe.tile([P, P], f32, tag="qe")
                nc.tensor.matmul(qe_ps[:], lhsT=selT[:], rhs=qtile[:],
                                 start=True, stop=True)

                keE = wkp.tile([P, P], f32, tag="keE")
                nc.vector.tensor_tensor(out=keE[:], in0=ke, in1=e_ps[:], op=ALU.mult)
                prod = wkp.tile([P, P], f32, tag="prod")
                nc.vector.tensor_tensor(out=prod[:], in0=qe_ps[:], in1=keE[:],
                                        op=ALU.mult)
                logit = smallp.tile([P, H], f32, tag="logit")
                nc.vector.tensor_reduce(
                    out=logit[:], in_=prod[:].rearrange("p (h d) -> p h d", h=H),
                    axis=mybir.AxisListType.X, op=ALU.add)
                nc.vector.tensor_scalar_min(out=logit[:], in0=logit[:], scalar1=5.0)
                nc.vector.tensor_scalar_max(out=logit[:], in0=logit[:], scalar1=-5.0)
                attn_j = attn_sb[:, j * H:(j + 1) * H]
                nc.scalar.activation(attn_j, logit[:], AF.Exp)

                msg = wkp.tile([P, P], f32, tag="msg")
                nc.vector.tensor_tensor(
                    out=msg[:].rearrange("p (h d) -> p h d", h=H),
                    in0=ve.rearrange("p (h d) -> p h d", h=H),
                    in1=attn_j.unsqueeze(-1).to_broadcast([P, H, DV]),
                    op=ALU.mult)

                nc.tensor.matmul(numT[:], lhsT=msg[:], rhs=sel[:],
                                 start=(j == 0), stop=(j == KB - 1))
                nc.tensor.matmul(denT[:], lhsT=attn_j, rhs=sel[:],
                                 start=(j == 0), stop=(j == KB - 1))
                if jj == SS - 1 or j == KB - 1:
                    # release dead supers so pool slots recycle
                    for sdead in [k for k in super_tiles if k < s]:
                        del super_tiles[sdead]

            # ---- finalize node-tile ----
            rden = smallp.tile([H, P], f32, tag="rden")
            nc.vector.tensor_scalar_add(out=rden[:], in0=denT[:], scalar1=1e-8)
            nc.vector.reciprocal(rden[:], rden[:])
            r_ps = ps_nt.tile([P, P], f32, tag="ntwork")
            nc.tensor.matmul(r_ps[:], lhsT=e4t_sb[:], rhs=rden[:],
                             start=True, stop=True)
            r_sb = ntp.tile([P, P], f32, tag="rsb")
            nc.scalar.activation(r_sb[:], r_ps[:], AF.Copy)
            nnorm = ntp.tile([P, P], f32, tag="nnorm")
            nc.vector.tensor_tensor(out=nnorm[:], in0=numT[:], in1=r_sb[:],
                                    op=ALU.mult)
            fc_ps = ps_nt.tile([P, P], f32, tag="ntwork")
            nc.tensor.matmul(fc_ps[:], lhsT=nnorm[:], rhs=w_sb["wfc"][:],
                             start=True, stop=True)
            y = ntp.tile([P, P], f32, tag="y")
            nc.vector.tensor_tensor(out=y[:], in0=fc_ps[:], in1=xq_t[:], op=ALU.add)
            # layernorm over free dim
            mu = smallp.tile([P, 1], f32, tag="mu")
            nc.vector.tensor_reduce(out=mu[:], in_=y[:],
                                    axis=mybir.AxisListType.X, op=ALU.add)
            nc.vector.tensor_scalar_mul(out=mu[:], in0=mu[:], scalar1=-1.0 / DM)
            xc = ntp.tile([P, P], f32, tag="xc")
            nc.vector.tensor_tensor(out=xc[:], in0=y[:],
                                    in1=mu[:].to_broadcast([P, P]), op=ALU.add)
            sq = wkp.tile([P, P], f32, tag="sq")
            nc.vector.tensor_tensor(out=sq[:], in0=xc[:], in1=xc[:], op=ALU.mult)
            var1 = smallp.tile([P, 1], f32, tag="var1")
            nc.vector.tensor_reduce(out=var1[:], in_=sq[:],
                                    axis=mybir.AxisListType.X, op=ALU.add)
            # var = var_raw / DM + eps, then std = sqrt(var)
            nc.vector.tensor_scalar(out=var1[:], in0=var1[:],
                                    scalar1=1.0 / DM, scalar2=1e-5,
                                    op0=ALU.mult, op1=ALU.add)
            std = smallp.tile([P, 1], f32, tag="std")
            nc.scalar.sqrt(std[:], var1[:])
            rstd = smallp.tile([P, 1], f32, tag="rstd")
            nc.vector.reciprocal(rstd[:], std[:])
            xn = ntp.tile([P, P], f32, tag="xn")
            nc.vector.tensor_tensor(out=xn[:], in0=xc[:],
                                    in1=rstd[:].to_broadcast([P, P]), op=ALU.mult)
            xg = ntp.tile([P, P], f32, tag="xg")
            nc.vector.tensor_tensor(out=xg[:], in0=xn[:], in1=gamma_mat[:],
                                    op=ALU.mult)
            y2 = ntp.tile([P, P], f32, tag="y2")
            nc.vector.tensor_tensor(out=y2[:], in0=xg[:], in1=beta_mat[:],
                                    op=ALU.add)
            nc.sync.dma_start(outp[nt_i * P:(nt_i + 1) * P, :], y2[:])
            nc.sync.dma_start(attn_o[nt_i], attn_sb[:])

        for _rep in range(repeat):
            super_tiles.clear()
            emit_phase1()
            emit_phase2()

    nc.compile()
    return nc


# ----------------------------------------------------------------------------
# Host-side sharding / unsharding
# ----------------------------------------------------------------------------

def plan_core(cfg: Cfg, src: np.ndarray, half: int):
    """Bucket this half's edges into balanced node-tiles.

    Returns (node_of_local, src_tl, eids) where node_of_local[l] is the
    half-local node id at kernel-local position l; src_tl[slot]/eids[slot]
    give the in-tile src position (255=pad) and original edge id (-1=pad)
    for each of the ecap edge slots in device order.
    """
    NH, NT, KB = cfg.nh, cfg.nt, cfg.kb
    m = (src >= half * NH) & (src < (half + 1) * NH)
    eids_all = np.nonzero(m)[0].astype(np.int64)
    loc = (src[eids_all] - half * NH).astype(np.int64)
    pn = np.bincount(loc, minlength=NH)
    order = np.argsort(-pn, kind="stable")
    bins_cnt = np.zeros(NT, np.int64)
    bins_n = np.zeros(NT, np.int64)
    node_bin = np.zeros(NH, np.int64)
    for node in order:
        avail = np.nonzero(bins_n < P)[0]
        b = avail[np.argmin(bins_cnt[avail])]
        node_bin[node] = b
        bins_cnt[b] += pn[node]
        bins_n[b] += 1
    assert bins_cnt.max() <= KB * P, f"bin overflow: {bins_cnt.max()} > {KB * P}"
    # position of each node within its bin
    node_pos = np.zeros(NH, np.int64)
    fill = np.zeros(NT, np.int64)
    node_of_local = np.zeros(NH, np.int64)
    for node in range(NH):
        b = node_bin[node]
        node_pos[node] = fill[b]
        node_of_local[b * P + fill[b]] = node
        fill[b] += 1
    # edge slots
    ecap = cfg.ecap
    src_tl = np.full(ecap, 255, np.int64)
    eids = np.full(ecap, -1, np.int64)
    ebin = node_bin[loc]
    order_e = np.argsort(ebin, kind="stable")
    eb_sorted = ebin[order_e]
    starts = np.searchsorted(eb_sorted, np.arange(NT))
    ends = np.searchsorted(eb_sorted, np.arange(NT) + 1)
    for b in range(NT):
        es = order_e[starts[b]:ends[b]]
        k = len(es)
        base = b * KB * P
        src_tl[base:base + k] = node_pos[loc[es]]
        eids[base:base + k] = eids_all[es]
    return node_of_local, src_tl, eids


def host_prep_core(cfg: Cfg, g_inputs: dict, half: int):
    """Build one core's input map. g_inputs holds one graph's arrays."""
    NH, NT, KB, SS = cfg.nh, cfg.nt, cfg.kb, cfg.ss
    SE, NSUP, NF = cfg.se, cfg.nsup, cfg.n_full
    src = g_inputs["src"]
    tgt = g_inputs["tgt"]
    node_of_local, src_tl, eids = plan_core(cfg, src, half)

    xq_g = g_inputs["input_Q"]  # [NF, DM]
    xq_half = xq_g[half * NH:(half + 1) * NH][node_of_local]
    eids_safe = np.where(eids < 0, 0, eids)
    tgt_slot = np.where(eids < 0, 0, tgt[eids_safe]).astype(np.int64)

    srcilv = src_tl.reshape(NT, KB, P).transpose(0, 2, 1).astype(np.float32)
    tgt16 = np.zeros((NSUP, 16, SE // 16), np.int16)
    t = tgt_slot.reshape(NSUP, SE)
    for i in range(SE):
        tgt16[:, i % 16, i // 16] = t[:, i]
    tgt16 = np.broadcast_to(tgt16[:, None, :, :], (NSUP, 8, 16, SE // 16)) \
        .reshape(NSUP, P, SE // 16).copy()

    efeat = g_inputs["edge_features"]  # [EG, DM]
    ef_rows = efeat[np.where(eids < 0, 0, eids_safe)]
    ef_rows = np.where((eids < 0)[:, None], 0.0, ef_rows).astype(np.float32)
    # [nsup, SS, P, DM] -> [nsup, DM, SS, P] (transposed per sub-block)
    ef = ef_rows.reshape(NSUP, SS, P, DM).transpose(0, 3, 1, 2) \
        .reshape(NSUP, DM, SS * P).copy()

    e4t = np.zeros((H, P), np.float32)
    for h in range(H):
        e4t[h, h * DV:(h + 1) * DV] = 1.0

    inv_sqrt_dk = np.float32(1.0 / np.sqrt(DK))
    in_map = {
        "xqT": np.ascontiguousarray(xq_half.T, np.float32),
        "xq": np.ascontiguousarray(xq_half, np.float32),
        "xkT": np.ascontiguousarray(g_inputs["input_K"].T, np.float32),
        "xvT": np.ascontiguousarray(g_inputs["input_V"].T, np.float32),
        "ef": ef,
        "tgt16": tgt16,
        "srcilv": srcilv,
        "wq": (g_inputs["W_Q"] * inv_sqrt_dk).astype(np.float32),
        "wk": g_inputs["W_K"].astype(np.float32),
        "wv": g_inputs["W_V"].astype(np.float32),
        "we": g_inputs["W_E"].astype(np.float32),
        "wfc": g_inputs["W_fc"].astype(np.float32),
        "e4t": e4t,
        "iota": np.arange(P, dtype=np.float32),
        "gamma": g_inputs["ln_gamma"].astype(np.float32),
        "beta": g_inputs["ln_beta"].astype(np.float32),
    }
    meta = {"node_of_local": node_of_local, "eids": eids}
    return in_map, meta


def host_post(cfg: Cfg, results, metas, B, EG):
    NH, NT, KB = cfg.nh, cfg.nt, cfg.kb
    N = cfg.n_full
    out = np.zeros((B, N, DM), np.float32)
    attn_last = np.zeros((H, EG), np.float32)
    for c in range(2 * B):
        g, half = c // 2, c % 2
        r = results[c]
        m = metas[c]
        o = r["outp"]  # [NH, 128] in local node order
        glob = half * NH + m["node_of_local"]
        out[g, glob, :] = o
        if g == B - 1:
            a = r["attn_o"].reshape(NT, P, KB, H)  # [nt, p, j, h]
            a = a.transpose(0, 2, 1, 3).reshape(cfg.ecap, H)  # slot-order
            valid = m["eids"] >= 0
            attn_last[:, m["eids"][valid]] = a[valid].T
    return out, attn_last.reshape(1, H, EG, 1)


def core_reference(cfg: Cfg, in_map: dict):
    """Numpy emulation of one core's device program (for sim validation)."""
    NH, NT, KB, SS = cfg.nh, cfg.nt, cfg.kb, cfg.ss
    SE, NSUP, NF = cfg.se, cfg.nsup, cfg.n_full
    xq = in_map["xq"]
    q = xq @ in_map["wq"]  # [NH,128] (wq pre-scaled)
    k = in_map["xkT"].T @ in_map["wk"]
    v = in_map["xvT"].T @ in_map["wv"]
    kv = np.concatenate([k, v], axis=1)  # [NF,256]
    # unwrap tgt16
    tgtw = in_map["tgt16"][:, :16, :]  # [NSUP,16,SE//16]
    tgt = np.zeros((NSUP, SE), np.int64)
    for i in range(SE):
        tgt[:, i] = tgtw[:, i % 16, i // 16]
    tgt = tgt.reshape(-1)
    ef = in_map["ef"].reshape(NSUP, DM, SS, P).transpose(0, 2, 3, 1) \
        .reshape(cfg.ecap, DM)
    e = ef @ in_map["we"]  # [ecap,128]
    srcilv = in_map["srcilv"].astype(np.int64)  # [NT,P,KB]
    src_tl = srcilv.transpose(0, 2, 1).reshape(cfg.ecap)
    attn_o = np.zeros((cfg.ecap, H), np.float32)
    outp = np.zeros((NH, P), np.float32)
    for nt_i in range(NT):
        num = np.zeros((P, P), np.float32)
        den = np.zeros((P, H), np.float32)
        for j in range(KB):
            sl = slice((nt_i * KB + j) * P, (nt_i * KB + j + 1) * P)
            st = src_tl[sl]
            sel = (st[:, None] == np.arange(P)[None, :]).astype(np.float32)
            qe = sel @ q[nt_i * P:(nt_i + 1) * P]
            kvg = kv[tgt[sl]]
            ke, ve = kvg[:, :P], kvg[:, P:]
            prod = qe * ke * e[sl]
            logit = np.clip(prod.reshape(P, H, DV).sum(-1), -5.0, 5.0)
            attn = np.exp(logit).astype(np.float32)
            attn_o[sl] = attn
            msg = ve * np.repeat(attn, DV, axis=1)
            num += sel.T @ msg
            den += sel.T @ attn
        nnorm = num / np.repeat(den + 1e-8, DV, axis=1)
        y = nnorm @ in_map["wfc"] + in_map["xq"][nt_i * P:(nt_i + 1) * P]
        mu = y.mean(-1, keepdims=True)
        var = ((y - mu) ** 2).mean(-1, keepdims=True)
        yn = (y - mu) / np.sqrt(var + 1e-5)
        outp[nt_i * P:(nt_i + 1) * P] = yn * in_map["gamma"] + in_map["beta"]
    a = attn_o.reshape(NT, KB, P, H).transpose(0, 2, 1, 3).reshape(NT, P, KB * H)
    return {"outp": outp, "attn_o": a}


_PROGRAM_CACHE = {}


def get_program(cfg: Cfg):
    if cfg not in _PROGRAM_CACHE:
        _PROGRAM_CACHE[cfg] = build_program(cfg)
    return _PROGRAM_CACHE[cfg]


def kernel(**inputs):
    cfg = FULL
    edge_indices = np.asarray(inputs["edge_indices"])
    B = edge_indices.shape[0]
    EG = edge_indices.shape[2]
    in_maps, metas = [], []
    for c in range(N_CORES):
        g, half = c // 2, c % 2
        g_inputs = {
            "src": np.asarray(edge_indices[g, 0]).astype(np.int64),
            "tgt": np.asarray(edge_indices[g, 1]).astype(np.int64),
            "edge_features": np.asarray(inputs["edge_features"][g]),
            "input_Q": np.asarray(inputs["input_Q"][g]),
            "input_K": np.asarray(inputs["input_K"][g]),
            "input_V": np.asarray(inputs["input_V"][g]),
            "W_Q": np.asarray(inputs["W_Q"]),
            "W_K": np.asarray(inputs["W_K"]),
            "W_V": np.asarray(inputs["W_V"]),
            "W_E": np.asarray(inputs["W_E"]),
            "W_fc": np.asarray(inputs["W_fc"]),
            "ln_gamma": np.asarray(inputs["ln_gamma"]),
            "ln_beta": np.asarray(inputs["ln_beta"]),
        }
        im, meta = host_prep_core(cfg, g_inputs, half)
        in_maps.append(im)
        metas.append(meta)
    nc = get_program(cfg)
    res = bass_utils.run_bass_kernel_spmd(nc, in_maps, core_ids=list(range(N_CORES)))
    out, attn_last = host_post(cfg, res.results, metas, B, EG)
    return out, attn_last


# revision 19
# speedup vs baseline: 1.0670x; 1.0670x over previous
"""CrossLocalAttentionLayer (GNN message passing) as a Bass/Tile kernel on 8 trn2 cores.

Strategy (data parallel over graphs + node-split within a graph):
  - 8 cores, B=4 graphs: core c handles graph c//2, node half c%2 (8192 nodes).
  - Edges are assigned to the core owning their src node, bucketed into 64
    balanced node-tiles of 128 nodes, padded to a uniform 9 sub-blocks of 128
    edges per node-tile (capacity 73728 edge slots per core).
  - On device, per core:
      phase 1: project K,V for the whole graph into a DRAM table [N,256].
      phase 2: per node-tile: project Q; per 1024-edge super-block, gather
        K/V rows by tgt with a hardware dma_gather; per 128-edge sub-block,
        project E, build the src one-hot selection matrix with is_equal
        against an iota matrix, gather Q rows with a PE matmul, compute
        attention logits/exp/messages with DVE+ACT, and accumulate
        numerator/denominator into PSUM with PE matmuls (segmented
        scatter-add as dense matmul against the selection matrix).
      finalize per node-tile: normalize, fc matmul, residual add, layernorm.
  - Host: bucketing/padding of indices, edge-feature redistribution in device
    order, inverse permutations on outputs.
"""

import os
import numpy as np
from contextlib import ExitStack
from dataclasses import dataclass

import concourse.bass as bass
import concourse.bacc as bacc
import concourse.mybir as mybir
import concourse.tile as tile
from concourse import bass_utils
from concourse.masks import make_identity

P = 128
F32 = mybir.dt.float32
I16 = mybir.dt.int16
AF = mybir.ActivationFunctionType
ALU = mybir.AluOpType

H, DK, DV, DM = 4, 32, 32, 128


@dataclass(frozen=True)
class Cfg:
    n_full: int = 16384   # nodes per graph
    nh: int = 8192        # nodes per core
    kb: int = 9           # 128-edge sub-blocks per node-tile
    ss: int = 8           # sub-blocks per gather super-block

    @property
    def nt(self):
        return self.nh // P

    @property
    def ecap(self):
        return self.nt * self.kb * P

    @property
    def se(self):
        return self.ss * P  # edges per super-block

    @property
    def nsup(self):
        return self.nt * self.kb // self.ss


FULL = Cfg()
N_CORES = 8
B_FULL, EG_FULL = 4, 131072


def build_program(cfg: Cfg, repeat: int = 1):
    """Build the single SPMD Bass program run on every core.

    repeat>1 duplicates the whole compute body (timing harness use only);
    outputs are simply overwritten on later repeats.
    """
    nc = bacc.Bacc("TRN2", target_bir_lowering=False, debug=False,
                   enable_asserts=False, num_devices=1)
    f32 = F32
    NF, NH, NT, KB, SS = cfg.n_full, cfg.nh, cfg.nt, cfg.kb, cfg.ss
    SE, NSUP = cfg.se, cfg.nsup

    din = {}
    def inp(name, shape, dt=f32):
        din[name] = nc.dram_tensor(name, shape, dt, kind="ExternalInput").ap()
        return din[name]

    xqT = inp("xqT", [P, NH])
    xq = inp("xq", [NH, P])
    xkT = inp("xkT", [P, NF])
    xvT = inp("xvT", [P, NF])
    ef = inp("ef", [NSUP, P, SS * P])
    tgt16 = inp("tgt16", [NSUP, P, SE // 16], I16)
    src16 = inp("src16", [NSUP, P, SE // 16], I16)
    srcilv = inp("srcilv", [NT, P, KB])
    wq = inp("wq", [P, P])
    wk = inp("wk", [P, P])
    wv = inp("wv", [P, P])
    we = inp("we", [P, P])
    wfc = inp("wfc", [P, P])
    e4t = inp("e4t", [H, P])
    iota = inp("iota", [P])
    gamma = inp("gamma", [P])
    beta = inp("beta", [P])

    outp = nc.dram_tensor("outp", [NH, P], f32, kind="ExternalOutput").ap()
    attn_o = nc.dram_tensor("attn_o", [NT, P, KB * H], f32, kind="ExternalOutput").ap()
    kvtab = nc.dram_tensor("kvtab", [NF, 2 * P], f32, kind="Internal").ap()
    qtab = nc.dram_tensor("qtab", [NH, P], f32, kind="Internal").ap()

    with tile.TileContext(nc) as tc, ExitStack() as ctx:
        consts = ctx.enter_context(tc.tile_pool(name="consts", bufs=1))
        p1 = ctx.enter_context(tc.tile_pool(name="p1", bufs=3))
        kvp = ctx.enter_context(tc.tile_pool(name="kvp", bufs=4))
        qgp = ctx.enter_context(tc.tile_pool(name="qgp", bufs=4))
        efp = ctx.enter_context(tc.tile_pool(name="efp", bufs=4))
        idxp = ctx.enter_context(tc.tile_pool(name="idxp", bufs=4))
        ntp = ctx.enter_context(tc.tile_pool(name="ntp", bufs=2))
        wkp = ctx.enter_context(tc.tile_pool(name="wkp", bufs=4))
        smallp = ctx.enter_context(tc.tile_pool(name="smallp", bufs=4))
        # PSUM pools: 8 banks total
        ps_acc = ctx.enter_context(tc.tile_pool(name="ps_acc", bufs=2, space="PSUM"))
        ps_den = ctx.enter_context(tc.tile_pool(name="ps_den", bufs=2, space="PSUM"))
        ps_ee = ctx.enter_context(tc.tile_pool(name="ps_ee", bufs=2, space="PSUM"))
        ps_nt = ctx.enter_context(tc.tile_pool(name="ps_nt", bufs=2, space="PSUM"))

        # ---- constants ----
        iota_mat = consts.tile([P, P], f32)
        nc.sync.dma_start(iota_mat[:], iota[None, :].to_broadcast([P, P]))
        gamma_mat = consts.tile([P, P], f32)
        nc.sync.dma_start(gamma_mat[:], gamma[None, :].to_broadcast([P, P]))
        beta_mat = consts.tile([P, P], f32)
        nc.sync.dma_start(beta_mat[:], beta[None, :].to_broadcast([P, P]))
        w_sb = {}
        for name, d in (("wq", wq), ("wk", wk), ("wv", wv), ("we", we), ("wfc", wfc)):
            t = consts.tile([P, P], f32, tag=name)
            nc.sync.dma_start(t[:], d[:])
            w_sb[name] = t
        e4t_sb = consts.tile([H, P], f32)
        nc.sync.dma_start(e4t_sb[:], e4t[:])

        # ---- phase 1: K/V and Q tables ----
        def emit_phase1():
          for i in range(NF // P):
            xk_t = p1.tile([P, P], f32, tag="xk")
            nc.sync.dma_start(xk_t[:], xkT[:, i * P:(i + 1) * P])
            xv_t = p1.tile([P, P], f32, tag="xv")
            nc.sync.dma_start(xv_t[:], xvT[:, i * P:(i + 1) * P])
            kv_ps = ps_nt.tile([P, 2 * P], f32, tag="ntwork")
            nc.tensor.matmul(kv_ps[:, 0:P], lhsT=xk_t[:], rhs=w_sb["wk"][:],
                             start=True, stop=True)
            nc.tensor.matmul(kv_ps[:, P:2 * P], lhsT=xv_t[:], rhs=w_sb["wv"][:],
                             start=True, stop=True)
            kv_sb = p1.tile([P, 2 * P], f32, tag="kvsb")
            nc.vector.tensor_copy(kv_sb[:], kv_ps[:])
            nc.sync.dma_start(kvtab[i * P:(i + 1) * P, :], kv_sb[:])
          for i in range(NH // P):
            xq_t = p1.tile([P, P], f32, tag="xqp1")
            nc.sync.dma_start(xq_t[:], xqT[:, i * P:(i + 1) * P])
            q_ps = ps_nt.tile([P, P], f32, tag="ntwork")
            nc.tensor.matmul(q_ps[:], lhsT=xq_t[:], rhs=w_sb["wq"][:],
                             start=True, stop=True)
            q_sb = p1.tile([P, P], f32, tag="qsb")
            nc.vector.tensor_copy(q_sb[:], q_ps[:])
            nc.sync.dma_start(qtab[i * P:(i + 1) * P, :], q_sb[:])

        # ---- phase 2 ----
        super_tiles = {}

        def ensure_super(s):
            if s in super_tiles or s >= NSUP:
                return
            idx_t = idxp.tile([P, SE // 16], I16, tag="idx")
            nc.sync.dma_start(idx_t[:], tgt16[s])
            kv_t = kvp.tile([P, SS, 2 * P], f32, tag="kvg")
            nc.gpsimd.dma_gather(
                out_ap=kv_t[:], in_ap=kvtab[:], idxs_ap=idx_t[:],
                num_idxs=SE, num_idxs_reg=SE, elem_size=2 * P)
            sidx_t = idxp.tile([P, SE // 16], I16, tag="sidx")
            nc.sync.dma_start(sidx_t[:], src16[s])
            q_t = qgp.tile([P, SS, P], f32, tag="qg")
            nc.gpsimd.dma_gather(
                out_ap=q_t[:], in_ap=qtab[:], idxs_ap=sidx_t[:],
                num_idxs=SE, num_idxs_reg=SE, elem_size=P)
            ef_t = efp.tile([P, SS * P], f32, tag="ef")
            nc.sync.dma_start(ef_t[:], ef[s])
            super_tiles[s] = (kv_t, q_t, ef_t)

        def emit_phase2():
          for nt_i in range(NT):
            srcilv_t = ntp.tile([P, KB], f32, tag="srcilv")
            nc.sync.dma_start(srcilv_t[:], srcilv[nt_i])
            xq_t = ntp.tile([P, P], f32, tag="xqres")
            nc.sync.dma_start(xq_t[:], xq[nt_i * P:(nt_i + 1) * P, :])
            attn_sb = ntp.tile([P, KB * H], f32, tag="attnsb")
            numT = ps_acc.tile([P, P], f32, tag="numT")
            denT = ps_den.tile([H, P], f32, tag="denT")

            for j in range(KB):
                sbi = nt_i * KB + j
                s, jj = divmod(sbi, SS)
                ensure_super(s)
                if jj == 0:
                    ensure_super(s + 1)
                kv_t, q_t, ef_t = super_tiles[s]
                ke = kv_t[:, jj, 0:P]
                ve = kv_t[:, jj, P:2 * P]
                qe = q_t[:, jj, :]

                e_ps = ps_ee.tile([P, P], f32, tag="ee")
                nc.tensor.matmul(e_ps[:], lhsT=ef_t[:, jj * P:(jj + 1) * P],
                                 rhs=w_sb["we"][:], start=True, stop=True)

                sel = wkp.tile([P, P], f32, tag="sel")
                nc.vector.tensor_tensor(
                    out=sel[:], in0=srcilv_t[:, j:j + 1].to_broadcast([P, P]),
                    in1=iota_mat[:], op=ALU.is_equal)

                keE = wkp.tile([P, P], f32, tag="keE")
                nc.vector.tensor_tensor(out=keE[:], in0=ke, in1=e_ps[:], op=ALU.mult)
                prod = wkp.tile([P, P], f32, tag="prod")
                nc.vector.tensor_tensor(out=prod[:], in0=qe, in1=keE[:],
                                        op=ALU.mult)
                logit = smallp.tile([P, H], f32, tag="logit")
                nc.vector.tensor_reduce(
                    out=logit[:], in_=prod[:].rearrange("p (h d) -> p h d", h=H),
                    axis=mybir.AxisListType.X, op=ALU.add)
                nc.vector.tensor_scalar_min(out=logit[:], in0=logit[:], scalar1=5.0)
                nc.vector.tensor_scalar_max(out=logit[:], in0=logit[:], scalar1=-5.0)
                attn_j = attn_sb[:, j * H:(j + 1) * H]
                nc.scalar.activation(attn_j, logit[:], AF.Exp)

                msg = wkp.tile([P, P], f32, tag="msg")
                nc.vector.tensor_tensor(
                    out=msg[:].rearrange("p (h d) -> p h d", h=H),
                    in0=ve.rearrange("p (h d) -> p h d", h=H),
                    in1=attn_j.unsqueeze(-1).to_broadcast([P, H, DV]),
                    op=ALU.mult)

                nc.tensor.matmul(numT[:], lhsT=msg[:], rhs=sel[:],
                                 start=(j == 0), stop=(j == KB - 1))
                nc.tensor.matmul(denT[:], lhsT=attn_j, rhs=sel[:],
                                 start=(j == 0), stop=(j == KB - 1))
                if jj == SS - 1 or j == KB - 1:
                    # release dead supers so pool slots recycle
                    for sdead in [k for k in super_tiles if k < s]:
                        del super_tiles[sdead]

            # ---- finalize node-tile ----
            rden = smallp.tile([H, P], f32, tag="rden")
            nc.vector.tensor_scalar_add(out=rden[:], in0=denT[:], scalar1=1e-8)
            nc.vector.reciprocal(rden[:], rden[:])
            r_ps = ps_nt.tile([P, P], f32, tag="ntwork")
            nc.tensor.matmul(r_ps[:], lhsT=e4t_sb[:], rhs=rden[:],
                             start=True, stop=True)
            r_sb = ntp.tile([P, P], f32, tag="rsb")
            nc.scalar.activation(r_sb[:], r_ps[:], AF.Copy)
            nnorm = ntp.tile([P, P], f32, tag="nnorm")
            nc.vector.tensor_tensor(out=nnorm[:], in0=numT[:], in1=r_sb[:],
                                    op=ALU.mult)
            fc_ps = ps_nt.tile([P, P], f32, tag="ntwork")
            nc.tensor.matmul(fc_ps[:], lhsT=nnorm[:], rhs=w_sb["wfc"][:],
                             start=True, stop=True)
            y = ntp.tile([P, P], f32, tag="y")
            nc.vector.tensor_tensor(out=y[:], in0=fc_ps[:], in1=xq_t[:], op=ALU.add)
            # layernorm over free dim
            mu = smallp.tile([P, 1], f32, tag="mu")
            nc.vector.tensor_reduce(out=mu[:], in_=y[:],
                                    axis=mybir.AxisListType.X, op=ALU.add)
            nc.vector.tensor_scalar_mul(out=mu[:], in0=mu[:], scalar1=-1.0 / DM)
            xc = ntp.tile([P, P], f32, tag="xc")
            nc.vector.tensor_tensor(out=xc[:], in0=y[:],
                                    in1=mu[:].to_broadcast([P, P]), op=ALU.add)
            sq = wkp.tile([P, P], f32, tag="sq")
            nc.vector.tensor_tensor(out=sq[:], in0=xc[:], in1=xc[:], op=ALU.mult)
            var1 = smallp.tile([P, 1], f32, tag="var1")
            nc.vector.tensor_reduce(out=var1[:], in_=sq[:],
                                    axis=mybir.AxisListType.X, op=ALU.add)
            # var = var_raw / DM + eps, then std = sqrt(var)
            nc.vector.tensor_scalar(out=var1[:], in0=var1[:],
                                    scalar1=1.0 / DM, scalar2=1e-5,
                                    op0=ALU.mult, op1=ALU.add)
            std = smallp.tile([P, 1], f32, tag="std")
            nc.scalar.sqrt(std[:], var1[:])
            rstd = smallp.tile([P, 1], f32, tag="rstd")
            nc.vector.reciprocal(rstd[:], std[:])
            xn = ntp.tile([P, P], f32, tag="xn")
            nc.vector.tensor_tensor(out=xn[:], in0=xc[:],
                                    in1=rstd[:].to_broadcast([P, P]), op=ALU.mult)
            xg = ntp.tile([P, P], f32, tag="xg")
            nc.vector.tensor_tensor(out=xg[:], in0=xn[:], in1=gamma_mat[:],
                                    op=ALU.mult)
            y2 = ntp.tile([P, P], f32, tag="y2")
            nc.vector.tensor_tensor(out=y2[:], in0=xg[:], in1=beta_mat[:],
                                    op=ALU.add)
            nc.sync.dma_start(outp[nt_i * P:(nt_i + 1) * P, :], y2[:])
            nc.sync.dma_start(attn_o[nt_i], attn_sb[:])

        for _rep in range(repeat):
            super_tiles.clear()
            emit_phase1()
            emit_phase2()

    nc.compile()
    return nc


# ----------------------------------------------------------------------------
# Host-side sharding / unsharding
# ----------------------------------------------------------------------------

def plan_core(cfg: Cfg, src: np.ndarray, half: int):
    """Bucket this half's edges into balanced node-tiles.

    Returns (node_of_local, src_tl, eids) where node_of_local[l] is the
    half-local node id at kernel-local position l; src_tl[slot]/eids[slot]
    give the in-tile src position (255=pad) and original edge id (-1=pad)
    for each of the ecap edge slots in device order.
    """
    NH, NT, KB = cfg.nh, cfg.nt, cfg.kb
    m = (src >= half * NH) & (src < (half + 1) * NH)
    eids_all = np.nonzero(m)[0].astype(np.int64)
    loc = (src[eids_all] - half * NH).astype(np.int64)
    pn = np.bincount(loc, minlength=NH)
    order = np.argsort(-pn, kind="stable")
    bins_cnt = np.zeros(NT, np.int64)
    bins_n = np.zeros(NT, np.int64)
    node_bin = np.zeros(NH, np.int64)
    for node in order:
        avail = np.nonzero(bins_n < P)[0]
        b = avail[np.argmin(bins_cnt[avail])]
        node_bin[node] = b
        bins_cnt[b] += pn[node]
        bins_n[b] += 1
    assert bins_cnt.max() <= KB * P, f"bin overflow: {bins_cnt.max()} > {KB * P}"
    # position of each node within its bin
    node_pos = np.zeros(NH, np.int64)
    fill = np.zeros(NT, np.int64)
    node_of_local = np.zeros(NH, np.int64)
    for node in range(NH):
        b = node_bin[node]
        node_pos[node] = fill[b]
        node_of_local[b * P + fill[b]] = node
        fill[b] += 1
    # edge slots
    ecap = cfg.ecap
    src_tl = np.full(ecap, 255, np.int64)
    eids = np.full(ecap, -1, np.int64)
    ebin = node_bin[loc]
    order_e = np.argsort(ebin, kind="stable")
    eb_sorted = ebin[order_e]
    starts = np.searchsorted(eb_sorted, np.arange(NT))
    ends = np.searchsorted(eb_sorted, np.arange(NT) + 1)
    for b in range(NT):
        es = order_e[starts[b]:ends[b]]
        k = len(es)
        base = b * KB * P
        src_tl[base:base + k] = node_pos[loc[es]]
        eids[base:base + k] = eids_all[es]
    return node_of_local, src_tl, eids


def host_prep_core(cfg: Cfg, g_inputs: dict, half: int):
    """Build one core's input map. g_inputs holds one graph's arrays."""
    NH, NT, KB, SS = cfg.nh, cfg.nt, cfg.kb, cfg.ss
    SE, NSUP, NF = cfg.se, cfg.nsup, cfg.n_full
    src = g_inputs["src"]
    tgt = g_inputs["tgt"]
    node_of_local, src_tl, eids = plan_core(cfg, src, half)

    xq_g = g_inputs["input_Q"]  # [NF, DM]
    xq_half = xq_g[half * NH:(half + 1) * NH][node_of_local]
    eids_safe = np.where(eids < 0, 0, eids)
    tgt_slot = np.where(eids < 0, 0, tgt[eids_safe]).astype(np.int64)

    srcilv = src_tl.reshape(NT, KB, P).transpose(0, 2, 1).astype(np.float32)

    def wrap16(vals):
        """[ecap] -> [NSUP, 128, SE//16] wrapped i16 layout for dma_gather."""
        w = np.zeros((NSUP, 16, SE // 16), np.int16)
        v = vals.reshape(NSUP, SE)
        for i in range(SE):
            w[:, i % 16, i // 16] = v[:, i]
        return np.broadcast_to(w[:, None, :, :], (NSUP, 8, 16, SE // 16)) \
            .reshape(NSUP, P, SE // 16).copy()

    tgt16 = wrap16(tgt_slot)
    # local q-row index per slot: node-tile base + in-tile position (pad -> 0)
    ntile_of_slot = np.repeat(np.arange(NT), KB * P)
    q_idx = np.where(src_tl == 255, 0, ntile_of_slot * P + src_tl)
    src16 = wrap16(q_idx)

    efeat = g_inputs["edge_features"]  # [EG, DM]
    ef_rows = efeat[np.where(eids < 0, 0, eids_safe)]
    ef_rows = np.where((eids < 0)[:, None], 0.0, ef_rows).astype(np.float32)
    # [nsup, SS, P, DM] -> [nsup, DM, SS, P] (transposed per sub-block)
    ef = ef_rows.reshape(NSUP, SS, P, DM).transpose(0, 3, 1, 2) \
        .reshape(NSUP, DM, SS * P).copy()

    e4t = np.zeros((H, P), np.float32)
    for h in range(H):
        e4t[h, h * DV:(h + 1) * DV] = 1.0

    inv_sqrt_dk = np.float32(1.0 / np.sqrt(DK))
    in_map = {
        "xqT": np.ascontiguousarray(xq_half.T, np.float32),
        "xq": np.ascontiguousarray(xq_half, np.float32),
        "xkT": np.ascontiguousarray(g_inputs["input_K"].T, np.float32),
        "xvT": np.ascontiguousarray(g_inputs["input_V"].T, np.float32),
        "ef": ef,
        "tgt16": tgt16,
        "src16": src16,
        "srcilv": srcilv,
        "wq": (g_inputs["W_Q"] * inv_sqrt_dk).astype(np.float32),
        "wk": g_inputs["W_K"].astype(np.float32),
        "wv": g_inputs["W_V"].astype(np.float32),
        "we": g_inputs["W_E"].astype(np.float32),
        "wfc": g_inputs["W_fc"].astype(np.float32),
        "e4t": e4t,
        "iota": np.arange(P, dtype=np.float32),
        "gamma": g_inputs["ln_gamma"].astype(np.float32),
        "beta": g_inputs["ln_beta"].astype(np.float32),
    }
    meta = {"node_of_local": node_of_local, "eids": eids}
    return in_map, meta


def host_post(cfg: Cfg, results, metas, B, EG):
    NH, NT, KB = cfg.nh, cfg.nt, cfg.kb
    N = cfg.n_full
    out = np.zeros((B, N, DM), np.float32)
    attn_last = np.zeros((H, EG), np.float32)
    for c in range(2 * B):
        g, half = c // 2, c % 2
        r = results[c]
        m = metas[c]
        o = r["outp"]  # [NH, 128] in local node order
        glob = half * NH + m["node_of_local"]
        out[g, glob, :] = o
        if g == B - 1:
            a = r["attn_o"].reshape(NT, P, KB, H)  # [nt, p, j, h]
            a = a.transpose(0, 2, 1, 3).reshape(cfg.ecap, H)  # slot-order
            valid = m["eids"] >= 0
            attn_last[:, m["eids"][valid]] = a[valid].T
    return out, attn_last.reshape(1, H, EG, 1)


def core_reference(cfg: Cfg, in_map: dict):
    """Numpy emulation of one core's device program (for sim validation)."""
    NH, NT, KB, SS = cfg.nh, cfg.nt, cfg.kb, cfg.ss
    SE, NSUP, NF = cfg.se, cfg.nsup, cfg.n_full
    xq = in_map["xq"]
    q = xq @ in_map["wq"]  # [NH,128] (wq pre-scaled)
    k = in_map["xkT"].T @ in_map["wk"]
    v = in_map["xvT"].T @ in_map["wv"]
    kv = np.concatenate([k, v], axis=1)  # [NF,256]
    # unwrap tgt16
    tgtw = in_map["tgt16"][:, :16, :]  # [NSUP,16,SE//16]
    tgt = np.zeros((NSUP, SE), np.int64)
    for i in range(SE):
        tgt[:, i] = tgtw[:, i % 16, i // 16]
    tgt = tgt.reshape(-1)
    ef = in_map["ef"].reshape(NSUP, DM, SS, P).transpose(0, 2, 3, 1) \
        .reshape(cfg.ecap, DM)
    e = ef @ in_map["we"]  # [ecap,128]
    srcilv = in_map["srcilv"].astype(np.int64)  # [NT,P,KB]
    src_tl = srcilv.transpose(0, 2, 1).reshape(cfg.ecap)
    attn_o = np.zeros((cfg.ecap, H), np.float32)
    outp = np.zeros((NH, P), np.float32)
    for nt_i in range(NT):
        num = np.zeros((P, P), np.float32)
        den = np.zeros((P, H), np.float32)
        for j in range(KB):
            sl = slice((nt_i * KB + j) * P, (nt_i * KB + j + 1) * P)
            st = src_tl[sl]
            sel = (st[:, None] == np.arange(P)[None, :]).astype(np.float32)
            qe = q[np.where(st == 255, 0, nt_i * P + st)]
            kvg = kv[tgt[sl]]
            ke, ve = kvg[:, :P], kvg[:, P:]
            prod = qe * ke * e[sl]
            logit = np.clip(prod.reshape(P, H, DV).sum(-1), -5.0, 5.0)
            attn = np.exp(logit).astype(np.float32)
            attn_o[sl] = attn
            msg = ve * np.repeat(attn, DV, axis=1)
            num += sel.T @ msg
            den += sel.T @ attn
        nnorm = num / np.repeat(den + 1e-8, DV, axis=1)
        y = nnorm @ in_map["wfc"] + in_map["xq"][nt_i * P:(nt_i + 1) * P]
        mu = y.mean(-1, keepdims=True)
        var = ((y - mu) ** 2).mean(-1, keepdims=True)
        yn = (y - mu) / np.sqrt(var + 1e-5)
        outp[nt_i * P:(nt_i + 1) * P] = yn * in_map["gamma"] + in_map["beta"]
    a = attn_o.reshape(NT, KB, P, H).transpose(0, 2, 1, 3).reshape(NT, P, KB * H)
    return {"outp": outp, "attn_o": a}


_PROGRAM_CACHE = {}


def get_program(cfg: Cfg):
    if cfg not in _PROGRAM_CACHE:
        _PROGRAM_CACHE[cfg] = build_program(cfg)
    return _PROGRAM_CACHE[cfg]


def kernel(**inputs):
    cfg = FULL
    edge_indices = np.asarray(inputs["edge_indices"])
    B = edge_indices.shape[0]
    EG = edge_indices.shape[2]
    in_maps, metas = [], []
    for c in range(N_CORES):
        g, half = c // 2, c % 2
        g_inputs = {
            "src": np.asarray(edge_indices[g, 0]).astype(np.int64),
            "tgt": np.asarray(edge_indices[g, 1]).astype(np.int64),
            "edge_features": np.asarray(inputs["edge_features"][g]),
            "input_Q": np.asarray(inputs["input_Q"][g]),
            "input_K": np.asarray(inputs["input_K"][g]),
            "input_V": np.asarray(inputs["input_V"][g]),
            "W_Q": np.asarray(inputs["W_Q"]),
            "W_K": np.asarray(inputs["W_K"]),
            "W_V": np.asarray(inputs["W_V"]),
            "W_E": np.asarray(inputs["W_E"]),
            "W_fc": np.asarray(inputs["W_fc"]),
            "ln_gamma": np.asarray(inputs["ln_gamma"]),
            "ln_beta": np.asarray(inputs["ln_beta"]),
        }
        im, meta = host_prep_core(cfg, g_inputs, half)
        in_maps.append(im)
        metas.append(meta)
    nc = get_program(cfg)
    res = bass_utils.run_bass_kernel_spmd(nc, in_maps, core_ids=list(range(N_CORES)))
    out, attn_last = host_post(cfg, res.results, metas, B, EG)
    return out, attn_last
